# revision 1
# baseline (speedup 1.0000x reference)
"""Trainium2 Bass kernel for the rank-weighted hard-negative hinge loss.

Math (reference):
    scores = im @ s.T                         # [N, N]
    diag   = diagonal(scores)
    rank1[i] = #{j : scores[i,j] < diag[i]}   (row rank of diag)
    rank2[j] = #{i : scores[i,j] < diag[j]}   (col rank of diag)
    cost_s  = 1/(rank1+1) * max_j!=i relu(M + scores[i,j] - diag[i])
    cost_im = 1/(rank2+1) * max_i!=j relu(M + scores[i,j] - diag[j])
    loss = sum(cost_s) + sum(cost_im)

Key identities used on-device:
    max_j relu(M + x_j - d) = relu(M + max_j x_j - d)   (relu/+const monotone)
so each core only needs, per row/column of its score block:
    - the masked row/col max of raw scores
    - the rank counts
Row counts come from an ACT Sign pass with per-partition bias and fused
free-dim accumulation; column counts come from a DVE is_lt compare (bf16
indicator) summed over partitions by a bf16 ones-matmul on the PE. The
diagonal is excluded by adding -1e30 to the (i,i) entries of the PSUM
score block; the masked entry then deterministically counts as "below
diag", which exactly yields rank+1 (= the weight denominator).

fp32 matmuls run at 4 cycles/row on the PE (float32r was measured tf32-class
and would flip rank comparisons), so the kernel computes the score block in
ONE orientation only; everything else is derived from that PSUM.

Sharding: core r owns rows [r*1024, (r+1)*1024). To keep a single SPMD
program, each core receives s.T with columns rotated left by r*1024 so the
diagonal block sits at local column offset = local row index on every core.
Column stats are un-rotated on the host, which also does the final (tiny)
reduction across cores, including the 128-partition colmax fold.
"""

import os
import numpy as np

N = 8192
D = 256
NCORES = 8
RL = N // NCORES  # rows per core
MARGIN = 0.2
NEG = np.float32(-1.0e30)

SC_W = 1024            # column superchunk width
NSC = N // SC_W        # 8 superchunks
NT = RL // 128         # 8 row tiles

_cache = {}


def _build_nc():
    import concourse.bacc as bacc
    import concourse.mybir as mybir
    from concourse.tile import TileContext

    f32 = mybir.dt.float32
    bf16 = mybir.dt.bfloat16

    Sign = mybir.ActivationFunctionType.Sign
    AX = mybir.AxisListType.X
    MAX = mybir.AluOpType.max
    ADD = mybir.AluOpType.add
    MULT = mybir.AluOpType.mult
    LT = mybir.AluOpType.is_lt

    nc = bacc.Bacc(None)

    imT = nc.declare_dram_parameter("imT", [D, RL], f32, isOutput=False)
    sT = nc.declare_dram_parameter("sT", [D, N], f32, isOutput=False)
    diag_r = nc.declare_dram_parameter("diag_r", [128, NT], f32, isOutput=False)
    diag_cb = nc.declare_dram_parameter("diag_cb", [128, N], f32, isOutput=False)
    negeye = nc.declare_dram_parameter("negeye", [128, 128], f32, isOutput=False)
    s1_o = nc.declare_dram_parameter("s1", [128, NT * NSC], f32, isOutput=True)
    rmax_o = nc.declare_dram_parameter("rmax", [128, NT * NSC], f32, isOutput=True)
    cnt2_o = nc.declare_dram_parameter("cnt2", [1, N], f32, isOutput=True)
    cmax_o = nc.declare_dram_parameter("cmax", [128, N], f32, isOutput=True)

    with TileContext(nc) as tc:
        with (
            tc.tile_pool(name="consts", bufs=1) as cpool,
            tc.tile_pool(name="data", bufs=1) as dpool,
            tc.tile_pool(name="ps", bufs=2, space="PSUM") as pspool,
            tc.tile_pool(name="pcnt", bufs=2, space="PSUM") as pcpool,
            tc.tile_pool(name="scratch", bufs=3) as tpool,
            tc.tile_pool(name="ind", bufs=3) as ipool,
            tc.tile_pool(name="outs", bufs=1) as opool,
        ):
            t_negeye = cpool.tile([128, 128], f32, tag="negeye")
            nc.sync.dma_start(out=t_negeye[:], in_=negeye[:])
            t_dr = cpool.tile([128, NT], f32, tag="dr")
            nc.sync.dma_start(out=t_dr[:], in_=diag_r[:])
            t_ones = cpool.tile([128, 1], bf16, tag="ones")
            nc.vector.memset(t_ones[:], 1.0)

            t_dcb = dpool.tile([128, N], f32, tag="dcb")
            nc.sync.dma_start(out=t_dcb[:], in_=diag_cb[:])

            t_imT = []
            for k in range(2):
                t = dpool.tile([128, RL], f32, tag=f"imT{k}")
                nc.sync.dma_start(out=t[:], in_=imT[k * 128:(k + 1) * 128, :])
                t_imT.append(t)
            t_sT = {}
            for b in range(NSC):
                for k in range(2):
                    t = dpool.tile([128, SC_W], f32, tag=f"sT{k}_{b}")
                    nc.sync.dma_start(
                        out=t[:],
                        in_=sT[k * 128:(k + 1) * 128, b * SC_W:(b + 1) * SC_W],
                    )
                    t_sT[(k, b)] = t

            t_s1 = opool.tile([128, NT * NSC], f32, tag="s1")
            t_rmax = opool.tile([128, NT * NSC], f32, tag="rmax")
            t_cnt2 = opool.tile([1, N], f32, tag="cnt2")
            t_cmax = opool.tile([128, N], f32, tag="cmax")
            nc.gpsimd.memset(t_cmax[:], -1.0e30)

            for sc in range(NSC):
                pc = pcpool.tile([1, SC_W], f32, tag="pcnt")
                for t in range(NT):
                    ps = pspool.tile([128, SC_W], f32, tag="ps")
                    for k in range(2):
                        for c in range(SC_W // 512):
                            nc.tensor.matmul(
                                ps[:, c * 512:(c + 1) * 512],
                                lhsT=t_imT[k][:, t * 128:(t + 1) * 128],
                                rhs=t_sT[(k, sc)][:, c * 512:(c + 1) * 512],
                                start=(k == 0),
                                stop=(k == 1),
                            )
                    if sc == 0:
                        off = t * 128
                        nc.vector.tensor_tensor(
                            ps[:, off:off + 128], ps[:, off:off + 128],
                            t_negeye[:], ADD,
                        )
                    # column indicator (scores < diag_col) -> bf16, feeds PE sum
                    ind = ipool.tile([128, SC_W], bf16, tag="ind")
                    nc.vector.scalar_tensor_tensor(
                        out=ind[:], in0=ps[:], scalar=1.0, in1=t_dcb[:, sc * SC_W:(sc + 1) * SC_W],
                        op0=MULT, op1=LT,
                    )
                    for c in range(SC_W // 512):
                        nc.tensor.matmul(
                            pc[0:1, c * 512:(c + 1) * 512],
                            lhsT=t_ones[:],
                            rhs=ind[:, c * 512:(c + 1) * 512],
                            start=(t == 0),
                            stop=(t == NT - 1),
                        )
                    idx = t * NSC + sc
                    trash = tpool.tile([128, SC_W], f32, tag="trash")
                    nc.scalar.activation(
                        trash[:], ps[:], Sign,
                        bias=t_dr[:, t:t + 1], scale=-1.0,
                        accum_out=t_s1[:, idx:idx + 1],
                    )
                    nc.vector.tensor_reduce(
                        t_rmax[:, idx:idx + 1], ps[:], AX, MAX,
                    )
                    nc.vector.tensor_tensor(
                        t_cmax[:, sc * SC_W:(sc + 1) * SC_W],
                        t_cmax[:, sc * SC_W:(sc + 1) * SC_W],
                        ps[:], MAX,
                    )
                nc.vector.tensor_copy(t_cnt2[0:1, sc * SC_W:(sc + 1) * SC_W], pc[0:1, :])

            nc.sync.dma_start(out=s1_o[:], in_=t_s1[:])
            nc.sync.dma_start(out=rmax_o[:], in_=t_rmax[:])
            nc.sync.dma_start(out=cnt2_o[:], in_=t_cnt2[:])
            nc.sync.dma_start(out=cmax_o[:], in_=t_cmax[:])

    nc.finalize()
    return nc


def _get_nc():
    if "nc" not in _cache:
        _cache["nc"] = _build_nc()
    return _cache["nc"]


def make_in_maps(im, s):
    im = np.ascontiguousarray(np.asarray(im, dtype=np.float32))
    s = np.ascontiguousarray(np.asarray(s, dtype=np.float32))
    diag = np.einsum("ij,ij->i", im, s).astype(np.float32)
    sT_full = np.ascontiguousarray(s.T)
    negeye = np.where(np.eye(128, dtype=bool), NEG, np.float32(0.0)).astype(np.float32)
    in_maps = []
    for r in range(NCORES):
        lo = r * RL
        rolled_diag = np.roll(diag, -lo)
        in_maps.append({
            "imT": np.ascontiguousarray(im[lo:lo + RL].T),
            "sT": np.ascontiguousarray(np.roll(sT_full, -lo, axis=1)),
            "diag_r": np.ascontiguousarray(diag[lo:lo + RL].reshape(NT, 128).T),
            "diag_cb": np.ascontiguousarray(
                np.broadcast_to(rolled_diag[None, :], (128, N))),
            "negeye": negeye,
        })
    return in_maps, diag


def finish(results, diag):
    """Host-side reduction of the per-core stats to the scalar loss."""
    diag64 = diag.astype(np.float64)
    total = 0.0
    cnt2_sum = np.zeros(N, dtype=np.float64)
    cmax_g = np.full(N, -np.inf, dtype=np.float64)
    for r in range(NCORES):
        lo = r * RL
        s1 = results[r]["s1"].astype(np.float64)        # [128, NT*NSC]
        rmax = results[r]["rmax"].astype(np.float64)
        cnt2 = results[r]["cnt2"].astype(np.float64)    # [1, N] counts
        cmax = results[r]["cmax"].astype(np.float64)    # [128, N]
        # s1/rmax: [128(p), t*NSC+sc] ; local row i = t*128 + p
        s1sum = s1.reshape(128, NT, NSC).sum(axis=2)
        rmax_row = rmax.reshape(128, NT, NSC).max(axis=2)
        cnt1 = (N + s1sum.T.reshape(RL)) / 2.0  # = rank1 + 1 (mask counts once)
        rmaxv = rmax_row.T.reshape(RL)
        d_loc = diag64[lo:lo + RL]
        total += np.sum(np.maximum(MARGIN + rmaxv - d_loc, 0.0) / cnt1)
        # columns: rotated col j' -> global j = (lo + j') % N
        jj = (lo + np.arange(N)) % N
        cnt2_sum[jj] += cnt2[0]
        cmax_g[jj] = np.maximum(cmax_g[jj], cmax.max(axis=0))
    cnt2_tot = cnt2_sum  # = rank2 + 1 (owning core's mask counts once)
    total += np.sum(np.maximum(MARGIN + cmax_g - diag64, 0.0) / cnt2_tot)
    return np.array(total, dtype=np.float32)


def run_on_hw(im, s, trace=False):
    from concourse.bass_utils import run_bass_kernel_spmd

    in_maps, diag = make_in_maps(im, s)
    nc = _get_nc()
    out = run_bass_kernel_spmd(nc, in_maps, list(range(NCORES)), trace=trace)
    return finish(out.results, diag), out


def kernel(im, s):
    result, _ = run_on_hw(im, s, trace=False)
    return result



# revision 13
# speedup vs baseline: 1.7472x; 1.7472x over previous
"""Trainium2 Bass kernel for the rank-weighted hard-negative hinge loss.

Math (reference):
    scores = im @ s.T                         # [N, N]
    diag   = diagonal(scores)
    rank1[i] = #{j : scores[i,j] < diag[i]}   (row rank of diag)
    rank2[j] = #{i : scores[i,j] < diag[j]}   (col rank of diag)
    cost_s  = 1/(rank1+1) * max_j!=i relu(M + scores[i,j] - diag[i])
    cost_im = 1/(rank2+1) * max_i!=j relu(M + scores[i,j] - diag[j])
    loss = sum(cost_s) + sum(cost_im)

v3 "dual orientation" design:
  - scores in fp16 (1 cyc/row on PE vs 4 for fp32; verified rel err ~6e-4
    against the 2e-2 gate, with the diagonal masked deterministically so
    rank counts are exact in structure: cnt = rank+1).
  - PE computes each score block TWICE: row-major ps[row, col] and
    transposed psT[col, row] (from the same SBUF-resident fp16 inputs).
    This makes BOTH rank counts per-partition-threshold Sign+accum ops
    on the ACT engine, removing the indicator tensors, count matmuls
    and the [128,N] diag broadcast of the fp32 baseline.
  - per block: DVE runs ONE fused tensor_tensor_reduce over ps that
    writes H = fp16(ps) to SBUF AND row-max-accumulates (rmax), then a
    2x-packed fp16 max accumulate for the running column max. rank1 for
    one of 8 row tiles runs as a 4x-packed tensor_scalar(is_lt)+accum
    on DVE to balance ACT vs DVE load. Pool does the t==0 column-max
    copies. Everything else is ACT Sign+accum.
  - host folds the tiny per-core stats (rank sums, rmax cols, cmax
    partitions) and reduces across the 8 cores.

Sharding: core r owns rows [r*1024, (r+1)*1024); s.T columns are rotated
left by r*1024 so the diagonal block sits at local column offset = local
row index on every core (single SPMD program). Host un-rotates.
"""

import numpy as np

N = 8192
D = 256
NCORES = 8
RL = N // NCORES  # rows per core
MARGIN = 0.2
MASK = np.float32(-30000.0)  # diag mask offset; fp16-safe sentinel

SC_W = 1024            # column superchunk width
NSC = N // SC_W        # 8 superchunks
NT = RL // 128         # 8 row tiles
NCC = N // 128         # 64 psT col chunks

_cache = {}


def _build_nc():
    import concourse.bacc as bacc
    import concourse.mybir as mybir
    from concourse.tile import TileContext

    f32 = mybir.dt.float32
    f16 = mybir.dt.float16

    Sign = mybir.ActivationFunctionType.Sign
    Copy = mybir.ActivationFunctionType.Copy
    AX = mybir.AxisListType.X
    MAX = mybir.AluOpType.max
    ADD = mybir.AluOpType.add
    MULT = mybir.AluOpType.mult
    LT = mybir.AluOpType.is_lt

    nc = bacc.Bacc(None)

    imT = nc.declare_dram_parameter("imT", [D, RL], f16, isOutput=False)
    sT = nc.declare_dram_parameter("sT", [D, N], f16, isOutput=False)
    diag_r = nc.declare_dram_parameter("diag_r", [128, NT], f32, isOutput=False)
    diag_c = nc.declare_dram_parameter("diag_c", [128, NCC], f32, isOutput=False)
    negeye = nc.declare_dram_parameter("negeye", [128, 128], f32, isOutput=False)
    s1_o = nc.declare_dram_parameter("s1", [128, NT * NSC], f32, isOutput=True)
    s2_o = nc.declare_dram_parameter("s2", [128, NCC], f32, isOutput=True)
    rmax_o = nc.declare_dram_parameter("rmax", [128, NT * NSC], f32, isOutput=True)
    cmax_o = nc.declare_dram_parameter("cmax", [128, N], f16, isOutput=True)

    with TileContext(nc) as tc:
        with (
            tc.tile_pool(name="consts", bufs=1) as cpool,
            tc.tile_pool(name="data", bufs=1) as dpool,
            tc.tile_pool(name="ps", bufs=2, space="PSUM") as pspool,
            tc.tile_pool(name="psT", bufs=2, space="PSUM") as pstpool,
            tc.tile_pool(name="hbuf", bufs=2) as hpool,
            tc.tile_pool(name="trash", bufs=3) as tpool,
            tc.tile_pool(name="outs", bufs=1) as opool,
        ):
            t_negeye = cpool.tile([128, 128], f32, tag="negeye")
            nc.sync.dma_start(out=t_negeye[:], in_=negeye[:])
            t_dr = cpool.tile([128, NT], f32, tag="dr")
            nc.sync.dma_start(out=t_dr[:], in_=diag_r[:])
            t_dc = cpool.tile([128, NCC], f32, tag="dc")
            nc.sync.dma_start(out=t_dc[:], in_=diag_c[:])
            t_ones = cpool.tile([128, SC_W], f16, tag="ones")
            nc.vector.memset(t_ones[:], 1.0)

            t_imT = []
            for k in range(2):
                t = dpool.tile([128, RL], f16, tag=f"imT{k}")
                nc.sync.dma_start(out=t[:], in_=imT[k * 128:(k + 1) * 128, :])
                t_imT.append(t)
            t_sT = {}
            for b in range(NSC):
                for k in range(2):
                    t = dpool.tile([128, SC_W], f16, tag=f"sT{k}_{b}")
                    nc.sync.dma_start(
                        out=t[:],
                        in_=sT[k * 128:(k + 1) * 128, b * SC_W:(b + 1) * SC_W],
                    )
                    t_sT[(k, b)] = t

            t_s1 = opool.tile([128, NT * NSC], f32, tag="s1")
            t_s2 = opool.tile([128, NCC], f32, tag="s2")
            t_rmax = opool.tile([128, NT * NSC], f32, tag="rmax")
            t_cmax = opool.tile([128, N], f16, tag="cmax")

            for sc in range(NSC):
                for t in range(NT):
                    idx = t * NSC + sc
                    cc = sc * NT + t  # psT col chunk handled this iteration

                    ps = pspool.tile([128, SC_W], f32, tag="ps")
                    for k in range(2):
                        for c in range(SC_W // 512):
                            nc.tensor.matmul(
                                ps[:, c * 512:(c + 1) * 512],
                                lhsT=t_imT[k][:, t * 128:(t + 1) * 128],
                                rhs=t_sT[(k, sc)][:, c * 512:(c + 1) * 512],
                                start=(k == 0),
                                stop=(k == 1),
                            )
                    psT = pstpool.tile([128, RL], f32, tag="psT")
                    for k in range(2):
                        for c in range(RL // 512):
                            nc.tensor.matmul(
                                psT[:, c * 512:(c + 1) * 512],
                                lhsT=t_sT[(k, sc)][:, t * 128:(t + 1) * 128],
                                rhs=t_imT[k][:, c * 512:(c + 1) * 512],
                                start=(k == 0),
                                stop=(k == 1),
                            )
                    if sc == 0:
                        off = t * 128
                        nc.vector.tensor_tensor(
                            ps[:, off:off + 128], ps[:, off:off + 128],
                            t_negeye[:], ADD,
                        )
                        # psT diag: col cc*128+p is at row (free) cc*128+p
                        nc.vector.tensor_tensor(
                            psT[:, off:off + 128], psT[:, off:off + 128],
                            t_negeye[:], ADD,
                        )
                    # H = fp16(ps) in SBUF (ACT copy; Pool cannot read PSUM
                    # and tensor_tensor_reduce faults at runtime here)
                    H = hpool.tile([128, SC_W], f16, tag="H")
                    nc.scalar.activation(H[:], ps[:], Copy)
                    # row max from H (frees ps early, SBUF-read rate)
                    nc.vector.tensor_reduce(
                        t_rmax[:, idx:idx + 1], H[:], AX, MAX)
                    # rank2: sign(diag_col - psT), accumulated over rows
                    trash2 = tpool.tile([128, SC_W], f16, tag="trash2")
                    nc.scalar.activation(
                        trash2[:], psT[:], Sign,
                        bias=t_dc[:, cc:cc + 1], scale=-1.0,
                        accum_out=t_s2[:, cc:cc + 1],
                    )
                    # rank1: mostly DVE (4x packed is_lt+accum on H, direct
                    # counts); one row tile per superchunk runs as ACT Sign
                    # on ps (sign sums) to balance ACT vs DVE load.
                    trash1 = tpool.tile([128, SC_W], f16, tag="trash1")
                    if t == 0:
                        nc.scalar.activation(
                            trash1[:], ps[:], Sign,
                            bias=t_dr[:, t:t + 1], scale=-1.0,
                            accum_out=t_s1[:, idx:idx + 1],
                        )
                    else:
                        # NOTE: with accum_out, op1 is the REDUCE op
                        # (TensorScalarPtrReduce): accum = add-reduce(out)
                        nc.vector.tensor_scalar(
                            trash1[:], H[:], t_dr[:, t:t + 1], 0.0, LT,
                            ADD, accum_out=t_s1[:, idx:idx + 1],
                        )
                    # running column max (fp16, 2x packed on DVE; Pool
                    # handles the initial copy)
                    cslice = t_cmax[:, sc * SC_W:(sc + 1) * SC_W]
                    if t == 0:
                        nc.gpsimd.tensor_copy(cslice, H[:])
                    else:
                        nc.vector.tensor_tensor(cslice, cslice, H[:], MAX)
                nc.sync.dma_start(
                    out=cmax_o[:, sc * SC_W:(sc + 1) * SC_W],
                    in_=t_cmax[:, sc * SC_W:(sc + 1) * SC_W])

            nc.sync.dma_start(out=s1_o[:], in_=t_s1[:])
            nc.sync.dma_start(out=s2_o[:], in_=t_s2[:])
            nc.sync.dma_start(out=rmax_o[:], in_=t_rmax[:])

    nc.finalize()
    return nc


def _get_nc():
    if "nc" not in _cache:
        _cache["nc"] = _build_nc()
    return _cache["nc"]


def make_in_maps(im, s):
    im = np.ascontiguousarray(np.asarray(im, dtype=np.float32))
    s = np.ascontiguousarray(np.asarray(s, dtype=np.float32))
    diag = np.einsum("ij,ij->i", im, s).astype(np.float32)
    imT16 = np.ascontiguousarray(im.T.astype(np.float16))
    sT16_full = np.ascontiguousarray(s.T.astype(np.float16))
    negeye = np.where(np.eye(128, dtype=bool), MASK, np.float32(0.0)).astype(
        np.float32)
    in_maps = []
    for r in range(NCORES):
        lo = r * RL
        rolled_diag = np.roll(diag, -lo)
        in_maps.append({
            "imT": np.ascontiguousarray(imT16[:, lo:lo + RL]),
            "sT": np.ascontiguousarray(np.roll(sT16_full, -lo, axis=1)),
            "diag_r": np.ascontiguousarray(diag[lo:lo + RL].reshape(NT, 128).T),
            "diag_c": np.ascontiguousarray(rolled_diag.reshape(NCC, 128).T),
            "negeye": negeye,
        })
    return in_maps, diag


def finish(results, diag):
    """Host-side reduction of the per-core stats to the scalar loss."""
    diag64 = diag.astype(np.float64)
    total = 0.0
    s2_sum = np.zeros(N, dtype=np.float64)
    cmax_g = np.full(N, -np.inf, dtype=np.float64)
    for r in range(NCORES):
        lo = r * RL
        s1 = results[r]["s1"].astype(np.float64)    # [128, NT*NSC]
        s2 = results[r]["s2"].astype(np.float64)    # [128, NCC] sign sums
        rmax = results[r]["rmax"].astype(np.float64)
        cmax = results[r]["cmax"].astype(np.float64)  # [128, N] (fp16 in)
        # s1: block (t, sc) in column t*NSC+sc. t==0 blocks hold ACT sign
        # sums -> (1024+S)/2; t>0 hold direct DVE is_lt counts.
        s1b = s1.reshape(128, NT, NSC)
        cnt_blk = np.empty_like(s1b)
        cnt_blk[:, 0, :] = (SC_W + s1b[:, 0, :]) / 2.0
        cnt_blk[:, 1:, :] = s1b[:, 1:, :]
        cnt1 = cnt_blk.sum(axis=2).T.reshape(RL)    # = rank1 + 1
        rmaxv = rmax.reshape(128, NT, NSC).max(axis=2).T.reshape(RL)
        d_loc = diag64[lo:lo + RL]
        total += np.sum(np.maximum(MARGIN + rmaxv - d_loc, 0.0) / cnt1)
        # columns: rotated col j' -> global j = (lo + j') % N
        jj = (lo + np.arange(N)) % N
        s2_sum[jj] += s2.T.reshape(N)               # sign sums over rows
        cmax_g[jj] = np.maximum(cmax_g[jj], cmax.max(axis=0))
    cnt2 = (N + s2_sum) / 2.0                       # = rank2 + 1
    total += np.sum(np.maximum(MARGIN + cmax_g - diag64, 0.0) / cnt2)
    return np.array(total, dtype=np.float32)


def run_on_hw(im, s, trace=False):
    from concourse.bass_utils import run_bass_kernel_spmd

    in_maps, diag = make_in_maps(im, s)
    nc = _get_nc()
    out = run_bass_kernel_spmd(nc, in_maps, list(range(NCORES)), trace=trace)
    return finish(out.results, diag), out


def kernel(im, s):
    result, _ = run_on_hw(im, s, trace=False)
    return result


# revision 17
# speedup vs baseline: 1.9476x; 1.1147x over previous
"""Trainium2 Bass kernel for the rank-weighted hard-negative hinge loss.

Math (reference):
    scores = im @ s.T                         # [N, N]
    diag   = diagonal(scores)
    rank1[i] = #{j : scores[i,j] < diag[i]}   (row rank of diag)
    rank2[j] = #{i : scores[i,j] < diag[j]}   (col rank of diag)
    cost_s  = 1/(rank1+1) * max_j!=i relu(M + scores[i,j] - diag[i])
    cost_im = 1/(rank2+1) * max_i!=j relu(M + scores[i,j] - diag[j])
    loss = sum(cost_s) + sum(cost_im)

v3 "dual orientation" design:
  - scores in fp16 (1 cyc/row on PE vs 4 for fp32; verified rel err ~6e-4
    against the 2e-2 gate, with the diagonal masked deterministically so
    rank counts are exact in structure: cnt = rank+1).
  - PE computes each score block TWICE: row-major ps[row, col] and
    transposed psT[col, row] (from the same SBUF-resident fp16 inputs).
    This makes BOTH rank counts per-partition-threshold Sign+accum ops
    on the ACT engine, removing the indicator tensors, count matmuls
    and the [128,N] diag broadcast of the fp32 baseline.
  - per block: DVE runs ONE fused tensor_tensor_reduce over ps that
    writes H = fp16(ps) to SBUF AND row-max-accumulates (rmax), then a
    2x-packed fp16 max accumulate for the running column max. rank1 for
    one of 8 row tiles runs as a 4x-packed tensor_scalar(is_lt)+accum
    on DVE to balance ACT vs DVE load. Pool does the t==0 column-max
    copies. Everything else is ACT Sign+accum.
  - host folds the tiny per-core stats (rank sums, rmax cols, cmax
    partitions) and reduces across the 8 cores.

Sharding: core r owns rows [r*1024, (r+1)*1024); s.T columns are rotated
left by r*1024 so the diagonal block sits at local column offset = local
row index on every core (single SPMD program). Host un-rotates.
"""

import numpy as np

N = 8192
D = 256
NCORES = 8
RL = N // NCORES  # rows per core
MARGIN = 0.2
MASK = np.float32(-30000.0)  # diag mask offset; fp16-safe sentinel

SC_W = 1024            # column superchunk width
NSC = N // SC_W        # 8 superchunks
NT = RL // 128         # 8 row tiles
NCC = N // 128         # 64 psT col chunks

_cache = {}


def _build_nc():
    import concourse.bacc as bacc
    import concourse.mybir as mybir
    from concourse.tile import TileContext

    f32 = mybir.dt.float32
    f16 = mybir.dt.float16

    Sign = mybir.ActivationFunctionType.Sign
    Copy = mybir.ActivationFunctionType.Copy
    AX = mybir.AxisListType.X
    MAX = mybir.AluOpType.max
    ADD = mybir.AluOpType.add
    MULT = mybir.AluOpType.mult
    LT = mybir.AluOpType.is_lt

    nc = bacc.Bacc(None)

    imT = nc.declare_dram_parameter("imT", [D, RL], f16, isOutput=False)
    sT = nc.declare_dram_parameter("sT", [D, N], f16, isOutput=False)
    diag_r = nc.declare_dram_parameter("diag_r", [128, NT], f32, isOutput=False)
    diag_c = nc.declare_dram_parameter("diag_c", [128, NCC], f32, isOutput=False)
    negeye = nc.declare_dram_parameter("negeye", [128, 128], f32, isOutput=False)
    s1_o = nc.declare_dram_parameter("s1", [128, NT * NSC], f32, isOutput=True)
    s2_o = nc.declare_dram_parameter("s2", [128, NCC], f32, isOutput=True)
    rmax_o = nc.declare_dram_parameter("rmax", [128, NT * NSC], f32, isOutput=True)
    cmax_o = nc.declare_dram_parameter("cmax", [128, N], f16, isOutput=True)

    with TileContext(nc) as tc:
        with (
            tc.tile_pool(name="consts", bufs=1) as cpool,
            tc.tile_pool(name="data", bufs=1) as dpool,
            tc.tile_pool(name="ps", bufs=2, space="PSUM") as pspool,
            tc.tile_pool(name="psT", bufs=2, space="PSUM") as pstpool,
            tc.tile_pool(name="hbuf", bufs=2) as hpool,
            tc.tile_pool(name="trash", bufs=3) as tpool,
            tc.tile_pool(name="outs", bufs=1) as opool,
        ):
            t_negeye = cpool.tile([128, 128], f32, tag="negeye")
            nc.sync.dma_start(out=t_negeye[:], in_=negeye[:])
            t_dr = cpool.tile([128, NT], f32, tag="dr")
            nc.sync.dma_start(out=t_dr[:], in_=diag_r[:])
            t_dc = cpool.tile([128, NCC], f32, tag="dc")
            nc.sync.dma_start(out=t_dc[:], in_=diag_c[:])
            t_ones = cpool.tile([128, SC_W], f16, tag="ones")
            nc.vector.memset(t_ones[:], 1.0)

            t_imT = []
            for k in range(2):
                t = dpool.tile([128, RL], f16, tag=f"imT{k}")
                nc.sync.dma_start(out=t[:], in_=imT[k * 128:(k + 1) * 128, :])
                t_imT.append(t)
            t_sT = {}
            for b in range(NSC):
                for k in range(2):
                    t = dpool.tile([128, SC_W], f16, tag=f"sT{k}_{b}")
                    nc.sync.dma_start(
                        out=t[:],
                        in_=sT[k * 128:(k + 1) * 128, b * SC_W:(b + 1) * SC_W],
                    )
                    t_sT[(k, b)] = t

            t_s1 = opool.tile([128, NT * NSC], f32, tag="s1")
            t_s2 = opool.tile([128, NCC], f32, tag="s2")
            t_rmax = opool.tile([128, NT * NSC], f32, tag="rmax")
            t_cmax = opool.tile([128, N], f16, tag="cmax")

            for sc in range(NSC):
                for t in range(NT):
                    idx = t * NSC + sc
                    cc = sc * NT + t  # psT col chunk handled this iteration

                    ps = pspool.tile([128, SC_W], f32, tag="ps")
                    for k in range(2):
                        for c in range(SC_W // 512):
                            nc.tensor.matmul(
                                ps[:, c * 512:(c + 1) * 512],
                                lhsT=t_imT[k][:, t * 128:(t + 1) * 128],
                                rhs=t_sT[(k, sc)][:, c * 512:(c + 1) * 512],
                                start=(k == 0),
                                stop=(k == 1),
                            )
                    psT = pstpool.tile([128, RL], f32, tag="psT")
                    for k in range(2):
                        for c in range(RL // 512):
                            nc.tensor.matmul(
                                psT[:, c * 512:(c + 1) * 512],
                                lhsT=t_sT[(k, sc)][:, t * 128:(t + 1) * 128],
                                rhs=t_imT[k][:, c * 512:(c + 1) * 512],
                                start=(k == 0),
                                stop=(k == 1),
                            )
                    if sc == 0:
                        off = t * 128
                        nc.vector.tensor_tensor(
                            ps[:, off:off + 128], ps[:, off:off + 128],
                            t_negeye[:], ADD,
                        )
                        # psT diag: col cc*128+p is at row (free) cc*128+p
                        nc.vector.tensor_tensor(
                            psT[:, off:off + 128], psT[:, off:off + 128],
                            t_negeye[:], ADD,
                        )
                    # row max straight from PSUM (no fp16 staging copy: the
                    # DVE 2x/4x packed modes do not engage on this HW, so a
                    # copy costs a full ACT pass and buys nothing)
                    nc.vector.tensor_reduce(
                        t_rmax[:, idx:idx + 1], ps[:], AX, MAX)
                    # rank2: sign(diag_col - psT), accumulated over rows
                    trash2 = tpool.tile([128, SC_W], f16, tag="trash2")
                    nc.scalar.activation(
                        trash2[:], psT[:], Sign,
                        bias=t_dc[:, cc:cc + 1], scale=-1.0,
                        accum_out=t_s2[:, cc:cc + 1],
                    )
                    # rank1: ACT Sign+accum on ps (sign sums)
                    trash1 = tpool.tile([128, SC_W], f16, tag="trash1")
                    nc.scalar.activation(
                        trash1[:], ps[:], Sign,
                        bias=t_dr[:, t:t + 1], scale=-1.0,
                        accum_out=t_s1[:, idx:idx + 1],
                    )
                    # running column max (fp16 accumulator in SBUF). ACT
                    # does the initial copy (gpsimd copy measured 3.6us);
                    # DVE accumulates the other seven row tiles.
                    cslice = t_cmax[:, sc * SC_W:(sc + 1) * SC_W]
                    if t == 0:
                        nc.scalar.activation(cslice, ps[:], Copy)
                    else:
                        nc.vector.tensor_tensor(cslice, cslice, ps[:], MAX)
                nc.sync.dma_start(
                    out=cmax_o[:, sc * SC_W:(sc + 1) * SC_W],
                    in_=t_cmax[:, sc * SC_W:(sc + 1) * SC_W])

            nc.sync.dma_start(out=s1_o[:], in_=t_s1[:])
            nc.sync.dma_start(out=s2_o[:], in_=t_s2[:])
            nc.sync.dma_start(out=rmax_o[:], in_=t_rmax[:])

    nc.finalize()
    return nc


def _get_nc():
    if "nc" not in _cache:
        _cache["nc"] = _build_nc()
    return _cache["nc"]


def make_in_maps(im, s):
    im = np.ascontiguousarray(np.asarray(im, dtype=np.float32))
    s = np.ascontiguousarray(np.asarray(s, dtype=np.float32))
    diag = np.einsum("ij,ij->i", im, s).astype(np.float32)
    imT16 = np.ascontiguousarray(im.T.astype(np.float16))
    sT16_full = np.ascontiguousarray(s.T.astype(np.float16))
    negeye = np.where(np.eye(128, dtype=bool), MASK, np.float32(0.0)).astype(
        np.float32)
    in_maps = []
    for r in range(NCORES):
        lo = r * RL
        rolled_diag = np.roll(diag, -lo)
        in_maps.append({
            "imT": np.ascontiguousarray(imT16[:, lo:lo + RL]),
            "sT": np.ascontiguousarray(np.roll(sT16_full, -lo, axis=1)),
            "diag_r": np.ascontiguousarray(diag[lo:lo + RL].reshape(NT, 128).T),
            "diag_c": np.ascontiguousarray(rolled_diag.reshape(NCC, 128).T),
            "negeye": negeye,
        })
    return in_maps, diag


def finish(results, diag):
    """Host-side reduction of the per-core stats to the scalar loss."""
    diag64 = diag.astype(np.float64)
    total = 0.0
    s2_sum = np.zeros(N, dtype=np.float64)
    cmax_g = np.full(N, -np.inf, dtype=np.float64)
    for r in range(NCORES):
        lo = r * RL
        s1 = results[r]["s1"].astype(np.float64)    # [128, NT*NSC]
        s2 = results[r]["s2"].astype(np.float64)    # [128, NCC] sign sums
        rmax = results[r]["rmax"].astype(np.float64)
        cmax = results[r]["cmax"].astype(np.float64)  # [128, N] (fp16 in)
        # s1: block (t, sc) in column t*NSC+sc holds ACT sign sums
        s1b = s1.reshape(128, NT, NSC)
        cnt_blk = (SC_W + s1b) / 2.0
        cnt1 = cnt_blk.sum(axis=2).T.reshape(RL)    # = rank1 + 1
        rmaxv = rmax.reshape(128, NT, NSC).max(axis=2).T.reshape(RL)
        d_loc = diag64[lo:lo + RL]
        total += np.sum(np.maximum(MARGIN + rmaxv - d_loc, 0.0) / cnt1)
        # columns: rotated col j' -> global j = (lo + j') % N
        jj = (lo + np.arange(N)) % N
        s2_sum[jj] += s2.T.reshape(N)               # sign sums over rows
        cmax_g[jj] = np.maximum(cmax_g[jj], cmax.max(axis=0))
    cnt2 = (N + s2_sum) / 2.0                       # = rank2 + 1
    total += np.sum(np.maximum(MARGIN + cmax_g - diag64, 0.0) / cnt2)
    return np.array(total, dtype=np.float32)


def run_on_hw(im, s, trace=False):
    from concourse.bass_utils import run_bass_kernel_spmd

    in_maps, diag = make_in_maps(im, s)
    nc = _get_nc()
    out = run_bass_kernel_spmd(nc, in_maps, list(range(NCORES)), trace=trace)
    return finish(out.results, diag), out


def kernel(im, s):
    result, _ = run_on_hw(im, s, trace=False)
    return result


# revision 23
# speedup vs baseline: 2.0729x; 1.0643x over previous
"""Trainium2 Bass kernel for the rank-weighted hard-negative hinge loss.

Math (reference):
    scores = im @ s.T                         # [N, N]
    diag   = diagonal(scores)
    rank1[i] = #{j : scores[i,j] < diag[i]}   (row rank of diag)
    rank2[j] = #{i : scores[i,j] < diag[j]}   (col rank of diag)
    cost_s  = 1/(rank1+1) * max_j!=i relu(M + scores[i,j] - diag[i])
    cost_im = 1/(rank2+1) * max_i!=j relu(M + scores[i,j] - diag[j])
    loss = sum(cost_s) + sum(cost_im)

v3 "dual orientation" design:
  - scores in fp16 (1 cyc/row on PE vs 4 for fp32; verified rel err ~6e-4
    against the 2e-2 gate, with the diagonal masked deterministically so
    rank counts are exact in structure: cnt = rank+1).
  - PE computes each score block TWICE: row-major ps[row, col] and
    transposed psT[col, row] (from the same SBUF-resident fp16 inputs).
    This makes BOTH rank counts per-partition-threshold Sign+accum ops
    on the ACT engine, removing the indicator tensors, count matmuls
    and the [128,N] diag broadcast of the fp32 baseline.
  - per block: DVE runs ONE fused tensor_tensor_reduce over ps that
    writes H = fp16(ps) to SBUF AND row-max-accumulates (rmax), then a
    2x-packed fp16 max accumulate for the running column max. rank1 for
    one of 8 row tiles runs as a 4x-packed tensor_scalar(is_lt)+accum
    on DVE to balance ACT vs DVE load. Pool does the t==0 column-max
    copies. Everything else is ACT Sign+accum.
  - host folds the tiny per-core stats (rank sums, rmax cols, cmax
    partitions) and reduces across the 8 cores.

Sharding: core r owns rows [r*1024, (r+1)*1024); s.T columns are rotated
left by r*1024 so the diagonal block sits at local column offset = local
row index on every core (single SPMD program). Host un-rotates.
"""

import numpy as np

N = 8192
D = 256
NCORES = 8
RL = N // NCORES  # rows per core
MARGIN = 0.2
MASK = np.float32(-30000.0)  # diag mask offset; fp16-safe sentinel

SC_W = 1024            # column superchunk width
NSC = N // SC_W        # 8 superchunks
NT = RL // 128         # 8 row tiles
NCC = N // 128         # 64 psT col chunks

_cache = {}


def _build_nc():
    import concourse.bacc as bacc
    import concourse.mybir as mybir
    from concourse.tile import TileContext

    f32 = mybir.dt.float32
    f16 = mybir.dt.float16

    Sign = mybir.ActivationFunctionType.Sign
    Copy = mybir.ActivationFunctionType.Copy
    AX = mybir.AxisListType.X
    MAX = mybir.AluOpType.max
    ADD = mybir.AluOpType.add
    MULT = mybir.AluOpType.mult
    LT = mybir.AluOpType.is_lt

    nc = bacc.Bacc(None)

    imT = nc.declare_dram_parameter("imT", [D, RL], f16, isOutput=False)
    sT = nc.declare_dram_parameter("sT", [D, N], f16, isOutput=False)
    diag_r = nc.declare_dram_parameter("diag_r", [128, NT], f32, isOutput=False)
    diag_c = nc.declare_dram_parameter("diag_c", [128, NCC], f32, isOutput=False)
    negeye = nc.declare_dram_parameter("negeye", [128, 128], f32, isOutput=False)
    s1_o = nc.declare_dram_parameter("s1", [128, NT * NSC], f32, isOutput=True)
    s2_o = nc.declare_dram_parameter("s2", [128, NCC], f32, isOutput=True)
    rmax_o = nc.declare_dram_parameter("rmax", [128, NT * NSC], f32, isOutput=True)
    cmax_o = nc.declare_dram_parameter("cmax", [128, NCC], f32, isOutput=True)

    with TileContext(nc) as tc:
        with (
            tc.tile_pool(name="consts", bufs=1) as cpool,
            tc.tile_pool(name="data", bufs=1) as dpool,
            tc.tile_pool(name="ps", bufs=2, space="PSUM") as pspool,
            tc.tile_pool(name="psT", bufs=2, space="PSUM") as pstpool,
            tc.tile_pool(name="hbuf", bufs=2) as hpool,
            tc.tile_pool(name="trash", bufs=3) as tpool,
            tc.tile_pool(name="outs", bufs=1) as opool,
        ):
            t_negeye = cpool.tile([128, 128], f32, tag="negeye")
            nc.sync.dma_start(out=t_negeye[:], in_=negeye[:])
            t_dr = cpool.tile([128, NT], f32, tag="dr")
            nc.sync.dma_start(out=t_dr[:], in_=diag_r[:])
            t_dc = cpool.tile([128, NCC], f32, tag="dc")
            nc.sync.dma_start(out=t_dc[:], in_=diag_c[:])
            t_ones = cpool.tile([128, SC_W], f16, tag="ones")
            nc.vector.memset(t_ones[:], 1.0)

            t_imT = []
            for k in range(2):
                t = dpool.tile([128, RL], f16, tag=f"imT{k}")
                nc.sync.dma_start(out=t[:], in_=imT[k * 128:(k + 1) * 128, :])
                t_imT.append(t)
            t_sT = {}
            for b in range(NSC):
                for k in range(2):
                    t = dpool.tile([128, SC_W], f16, tag=f"sT{k}_{b}")
                    nc.sync.dma_start(
                        out=t[:],
                        in_=sT[k * 128:(k + 1) * 128, b * SC_W:(b + 1) * SC_W],
                    )
                    t_sT[(k, b)] = t

            t_s1 = opool.tile([128, NT * NSC], f32, tag="s1")
            t_s2 = opool.tile([128, NCC], f32, tag="s2")
            t_rmax = opool.tile([128, NT * NSC], f32, tag="rmax")
            t_cmax = opool.tile([128, NCC], f32, tag="cmax")

            for sc in range(NSC):
                for t in range(NT):
                    idx = t * NSC + sc
                    cc = sc * NT + t  # psT col chunk handled this iteration

                    ps = pspool.tile([128, SC_W], f32, tag="ps")
                    for k in range(2):
                        for c in range(SC_W // 512):
                            nc.tensor.matmul(
                                ps[:, c * 512:(c + 1) * 512],
                                lhsT=t_imT[k][:, t * 128:(t + 1) * 128],
                                rhs=t_sT[(k, sc)][:, c * 512:(c + 1) * 512],
                                start=(k == 0),
                                stop=(k == 1),
                            )
                    psT = pstpool.tile([128, RL], f32, tag="psT")
                    for k in range(2):
                        for c in range(RL // 512):
                            nc.tensor.matmul(
                                psT[:, c * 512:(c + 1) * 512],
                                lhsT=t_sT[(k, sc)][:, t * 128:(t + 1) * 128],
                                rhs=t_imT[k][:, c * 512:(c + 1) * 512],
                                start=(k == 0),
                                stop=(k == 1),
                            )
                    if sc == 0:
                        off = t * 128
                        nc.vector.tensor_tensor(
                            ps[:, off:off + 128], ps[:, off:off + 128],
                            t_negeye[:], ADD,
                        )
                        # psT diag: col cc*128+p is at row (free) cc*128+p
                        nc.vector.tensor_tensor(
                            psT[:, off:off + 128], psT[:, off:off + 128],
                            t_negeye[:], ADD,
                        )
                    # row max straight from PSUM (no fp16 staging copy: the
                    # DVE 2x/4x packed modes do not engage on this HW, so a
                    # copy costs a full ACT pass and buys nothing)
                    nc.vector.tensor_reduce(
                        t_rmax[:, idx:idx + 1], ps[:], AX, MAX)
                    # rank2: sign(diag_col - psT), accumulated over rows
                    trash2 = tpool.tile([128, SC_W], f16, tag="trash2")
                    nc.scalar.activation(
                        trash2[:], psT[:], Sign,
                        bias=t_dc[:, cc:cc + 1], scale=-1.0,
                        accum_out=t_s2[:, cc:cc + 1],
                    )
                    # rank1: ACT Sign+accum on ps (sign sums); the t==0
                    # tile runs on DVE (is_lt + add-reduce accum -> direct
                    # count) to balance ACT vs DVE load.
                    trash1 = tpool.tile([128, SC_W], f16, tag="trash1")
                    if t == 0:
                        nc.vector.tensor_scalar(
                            trash1[:], ps[:], t_dr[:, t:t + 1], 0.0, LT,
                            ADD, accum_out=t_s1[:, idx:idx + 1],
                        )
                    else:
                        nc.scalar.activation(
                            trash1[:], ps[:], Sign,
                            bias=t_dr[:, t:t + 1], scale=-1.0,
                            accum_out=t_s1[:, idx:idx + 1],
                        )
                    # column max over this core's rows, straight from the
                    # transposed block (free-axis reduce, [128,1] per chunk)
                    nc.vector.tensor_reduce(
                        t_cmax[:, cc:cc + 1], psT[:], AX, MAX)

            nc.sync.dma_start(out=s1_o[:], in_=t_s1[:])
            nc.sync.dma_start(out=s2_o[:], in_=t_s2[:])
            nc.sync.dma_start(out=rmax_o[:], in_=t_rmax[:])
            nc.sync.dma_start(out=cmax_o[:], in_=t_cmax[:])

    nc.finalize()
    return nc


def _get_nc():
    if "nc" not in _cache:
        _cache["nc"] = _build_nc()
    return _cache["nc"]


def make_in_maps(im, s):
    im = np.ascontiguousarray(np.asarray(im, dtype=np.float32))
    s = np.ascontiguousarray(np.asarray(s, dtype=np.float32))
    diag = np.einsum("ij,ij->i", im, s).astype(np.float32)
    imT16 = np.ascontiguousarray(im.T.astype(np.float16))
    sT16_full = np.ascontiguousarray(s.T.astype(np.float16))
    negeye = np.where(np.eye(128, dtype=bool), MASK, np.float32(0.0)).astype(
        np.float32)
    in_maps = []
    for r in range(NCORES):
        lo = r * RL
        rolled_diag = np.roll(diag, -lo)
        in_maps.append({
            "imT": np.ascontiguousarray(imT16[:, lo:lo + RL]),
            "sT": np.ascontiguousarray(np.roll(sT16_full, -lo, axis=1)),
            "diag_r": np.ascontiguousarray(diag[lo:lo + RL].reshape(NT, 128).T),
            "diag_c": np.ascontiguousarray(rolled_diag.reshape(NCC, 128).T),
            "negeye": negeye,
        })
    return in_maps, diag


def finish(results, diag):
    """Host-side reduction of the per-core stats to the scalar loss."""
    diag64 = diag.astype(np.float64)
    total = 0.0
    s2_sum = np.zeros(N, dtype=np.float64)
    cmax_g = np.full(N, -np.inf, dtype=np.float64)
    for r in range(NCORES):
        lo = r * RL
        s1 = results[r]["s1"].astype(np.float64)    # [128, NT*NSC]
        s2 = results[r]["s2"].astype(np.float64)    # [128, NCC] sign sums
        rmax = results[r]["rmax"].astype(np.float64)
        cmax = results[r]["cmax"].astype(np.float64)  # [128, N] (fp16 in)
        # s1: block (t, sc) in column t*NSC+sc. t==0 blocks hold direct
        # DVE is_lt counts; t>0 hold ACT sign sums -> (1024+S)/2.
        s1b = s1.reshape(128, NT, NSC)
        cnt_blk = np.empty_like(s1b)
        cnt_blk[:, 0, :] = s1b[:, 0, :]
        cnt_blk[:, 1:, :] = (SC_W + s1b[:, 1:, :]) / 2.0
        cnt1 = cnt_blk.sum(axis=2).T.reshape(RL)    # = rank1 + 1
        rmaxv = rmax.reshape(128, NT, NSC).max(axis=2).T.reshape(RL)
        d_loc = diag64[lo:lo + RL]
        total += np.sum(np.maximum(MARGIN + rmaxv - d_loc, 0.0) / cnt1)
        # columns: rotated col j' = cc*128+p -> global j = (lo + j') % N
        jj = (lo + np.arange(N)) % N
        s2_sum[jj] += s2.T.reshape(N)               # sign sums over rows
        cmax_g[jj] = np.maximum(cmax_g[jj], cmax.T.reshape(N))
    cnt2 = (N + s2_sum) / 2.0                       # = rank2 + 1
    total += np.sum(np.maximum(MARGIN + cmax_g - diag64, 0.0) / cnt2)
    return np.array(total, dtype=np.float32)


def run_on_hw(im, s, trace=False):
    from concourse.bass_utils import run_bass_kernel_spmd

    in_maps, diag = make_in_maps(im, s)
    nc = _get_nc()
    out = run_bass_kernel_spmd(nc, in_maps, list(range(NCORES)), trace=trace)
    return finish(out.results, diag), out


def kernel(im, s):
    result, _ = run_on_hw(im, s, trace=False)
    return result


# revision 27
# speedup vs baseline: 2.0941x; 1.0103x over previous
"""Trainium2 Bass kernel for the rank-weighted hard-negative hinge loss.

Math (reference):
    scores = im @ s.T                         # [N, N]
    diag   = diagonal(scores)
    rank1[i] = #{j : scores[i,j] < diag[i]}   (row rank of diag)
    rank2[j] = #{i : scores[i,j] < diag[j]}   (col rank of diag)
    cost_s  = 1/(rank1+1) * max_j!=i relu(M + scores[i,j] - diag[i])
    cost_im = 1/(rank2+1) * max_i!=j relu(M + scores[i,j] - diag[j])
    loss = sum(cost_s) + sum(cost_im)

v3 "dual orientation" design:
  - scores in fp16 (1 cyc/row on PE vs 4 for fp32; verified rel err ~6e-4
    against the 2e-2 gate, with the diagonal masked deterministically so
    rank counts are exact in structure: cnt = rank+1).
  - PE computes each score block TWICE: row-major ps[row, col] and
    transposed psT[col, row] (from the same SBUF-resident fp16 inputs).
    This makes BOTH rank counts per-partition-threshold Sign+accum ops
    on the ACT engine, removing the indicator tensors, count matmuls
    and the [128,N] diag broadcast of the fp32 baseline.
  - per block: DVE runs ONE fused tensor_tensor_reduce over ps that
    writes H = fp16(ps) to SBUF AND row-max-accumulates (rmax), then a
    2x-packed fp16 max accumulate for the running column max. rank1 for
    one of 8 row tiles runs as a 4x-packed tensor_scalar(is_lt)+accum
    on DVE to balance ACT vs DVE load. Pool does the t==0 column-max
    copies. Everything else is ACT Sign+accum.
  - host folds the tiny per-core stats (rank sums, rmax cols, cmax
    partitions) and reduces across the 8 cores.

Sharding: core r owns rows [r*1024, (r+1)*1024); s.T columns are rotated
left by r*1024 so the diagonal block sits at local column offset = local
row index on every core (single SPMD program). Host un-rotates.
"""

import numpy as np

N = 8192
D = 256
NCORES = 8
RL = N // NCORES  # rows per core
MARGIN = 0.2
MASK = np.float32(-30000.0)  # diag mask offset; fp16-safe sentinel

SC_W = 1024            # column superchunk width
NSC = N // SC_W        # 8 superchunks
NT = RL // 128         # 8 row tiles
NCC = N // 128         # 64 psT col chunks

_cache = {}


def _build_nc():
    import concourse.bacc as bacc
    import concourse.mybir as mybir
    from concourse.tile import TileContext

    f32 = mybir.dt.float32
    f16 = mybir.dt.float16

    Sign = mybir.ActivationFunctionType.Sign
    Copy = mybir.ActivationFunctionType.Copy
    AX = mybir.AxisListType.X
    MAX = mybir.AluOpType.max
    ADD = mybir.AluOpType.add
    MULT = mybir.AluOpType.mult
    LT = mybir.AluOpType.is_lt

    nc = bacc.Bacc(None)

    imT = nc.declare_dram_parameter("imT", [D, RL], f16, isOutput=False)
    sT = nc.declare_dram_parameter("sT", [D, N], f16, isOutput=False)
    diag_r = nc.declare_dram_parameter("diag_r", [128, NT], f32, isOutput=False)
    diag_c = nc.declare_dram_parameter("diag_c", [128, NCC], f32, isOutput=False)
    eye16 = nc.declare_dram_parameter("eye16", [128, 128], f16, isOutput=False)
    negeye16 = nc.declare_dram_parameter("negeye16", [128, 128], f16, isOutput=False)
    s1_o = nc.declare_dram_parameter("s1", [128, NT * NSC], f32, isOutput=True)
    s2_o = nc.declare_dram_parameter("s2", [128, NCC], f32, isOutput=True)
    rmax_o = nc.declare_dram_parameter("rmax", [128, NT * NSC], f32, isOutput=True)
    cmax_o = nc.declare_dram_parameter("cmax", [128, NCC], f32, isOutput=True)

    with TileContext(nc) as tc:
        with (
            tc.tile_pool(name="consts", bufs=1) as cpool,
            tc.tile_pool(name="data", bufs=1) as dpool,
            tc.tile_pool(name="ps", bufs=2, space="PSUM") as pspool,
            tc.tile_pool(name="psT", bufs=2, space="PSUM") as pstpool,
            tc.tile_pool(name="hbuf", bufs=2) as hpool,
            tc.tile_pool(name="trash", bufs=3) as tpool,
            tc.tile_pool(name="outs", bufs=1) as opool,
        ):
            t_eye16 = cpool.tile([128, 128], f16, tag="eye16")
            nc.sync.dma_start(out=t_eye16[:], in_=eye16[:])
            t_negeye16 = cpool.tile([128, 128], f16, tag="negeye16")
            nc.sync.dma_start(out=t_negeye16[:], in_=negeye16[:])
            t_dr = cpool.tile([128, NT], f32, tag="dr")
            nc.sync.dma_start(out=t_dr[:], in_=diag_r[:])
            t_dc = cpool.tile([128, NCC], f32, tag="dc")
            nc.sync.dma_start(out=t_dc[:], in_=diag_c[:])
            t_ones = cpool.tile([128, SC_W], f16, tag="ones")
            nc.vector.memset(t_ones[:], 1.0)

            t_imT = []
            for k in range(2):
                t = dpool.tile([128, RL], f16, tag=f"imT{k}")
                nc.sync.dma_start(out=t[:], in_=imT[k * 128:(k + 1) * 128, :])
                t_imT.append(t)
            t_sT = {}
            for b in range(NSC):
                for k in range(2):
                    t = dpool.tile([128, SC_W], f16, tag=f"sT{k}_{b}")
                    nc.sync.dma_start(
                        out=t[:],
                        in_=sT[k * 128:(k + 1) * 128, b * SC_W:(b + 1) * SC_W],
                    )
                    t_sT[(k, b)] = t

            t_s1 = opool.tile([128, NT * NSC], f32, tag="s1")
            t_s2 = opool.tile([128, NCC], f32, tag="s2")
            t_rmax = opool.tile([128, NT * NSC], f32, tag="rmax")
            t_cmax = opool.tile([128, NCC], f32, tag="cmax")

            for sc in range(NSC):
                for t in range(NT):
                    idx = t * NSC + sc
                    cc = sc * NT + t  # psT col chunk handled this iteration

                    # sc==0 blocks contain the (rotated) diagonal at free
                    # offset t*128; mask it with an extra accumulating
                    # matmul  ps += I^T @ (-30000*I)  inside the group so
                    # no vector-engine pass (or extra dependency) is needed.
                    off = t * 128
                    cm = off // 512  # 512-wide region holding the diagonal
                    ps = pspool.tile([128, SC_W], f32, tag="ps")
                    for k in range(2):
                        for c in range(SC_W // 512):
                            nc.tensor.matmul(
                                ps[:, c * 512:(c + 1) * 512],
                                lhsT=t_imT[k][:, t * 128:(t + 1) * 128],
                                rhs=t_sT[(k, sc)][:, c * 512:(c + 1) * 512],
                                start=(k == 0),
                                stop=(k == 1) and not (sc == 0 and c == cm),
                            )
                    if sc == 0:
                        nc.tensor.matmul(
                            ps[:, off:off + 128],
                            lhsT=t_eye16[:], rhs=t_negeye16[:],
                            start=False, stop=True, skip_group_check=True,
                        )
                    psT = pstpool.tile([128, RL], f32, tag="psT")
                    for k in range(2):
                        for c in range(RL // 512):
                            nc.tensor.matmul(
                                psT[:, c * 512:(c + 1) * 512],
                                lhsT=t_sT[(k, sc)][:, t * 128:(t + 1) * 128],
                                rhs=t_imT[k][:, c * 512:(c + 1) * 512],
                                start=(k == 0),
                                stop=(k == 1) and not (sc == 0 and c == cm),
                            )
                    if sc == 0:
                        # psT diag: col cc*128+p is at row (free) cc*128+p
                        nc.tensor.matmul(
                            psT[:, off:off + 128],
                            lhsT=t_eye16[:], rhs=t_negeye16[:],
                            start=False, stop=True, skip_group_check=True,
                        )
                    # row max straight from PSUM (no fp16 staging copy: the
                    # DVE 2x/4x packed modes do not engage on this HW, so a
                    # copy costs a full ACT pass and buys nothing)
                    nc.vector.tensor_reduce(
                        t_rmax[:, idx:idx + 1], ps[:], AX, MAX)
                    # rank2: sign(diag_col - psT), accumulated over rows
                    trash2 = tpool.tile([128, SC_W], f16, tag="trash2")
                    nc.scalar.activation(
                        trash2[:], psT[:], Sign,
                        bias=t_dc[:, cc:cc + 1], scale=-1.0,
                        accum_out=t_s2[:, cc:cc + 1],
                    )
                    # rank1: ACT Sign+accum on ps (sign sums); the t==0
                    # tile runs on DVE (is_lt + add-reduce accum -> direct
                    # count) to balance ACT vs DVE load.
                    trash1 = tpool.tile([128, SC_W], f16, tag="trash1")
                    if t == 0:
                        nc.vector.tensor_scalar(
                            trash1[:], ps[:], t_dr[:, t:t + 1], 0.0, LT,
                            ADD, accum_out=t_s1[:, idx:idx + 1],
                        )
                    else:
                        nc.scalar.activation(
                            trash1[:], ps[:], Sign,
                            bias=t_dr[:, t:t + 1], scale=-1.0,
                            accum_out=t_s1[:, idx:idx + 1],
                        )
                    # column max over this core's rows, straight from the
                    # transposed block (free-axis reduce, [128,1] per chunk)
                    nc.vector.tensor_reduce(
                        t_cmax[:, cc:cc + 1], psT[:], AX, MAX)

            nc.sync.dma_start(out=s1_o[:], in_=t_s1[:])
            nc.sync.dma_start(out=s2_o[:], in_=t_s2[:])
            nc.sync.dma_start(out=rmax_o[:], in_=t_rmax[:])
            nc.sync.dma_start(out=cmax_o[:], in_=t_cmax[:])

    nc.finalize()
    return nc


def _get_nc():
    if "nc" not in _cache:
        _cache["nc"] = _build_nc()
    return _cache["nc"]


def make_in_maps(im, s):
    im = np.ascontiguousarray(np.asarray(im, dtype=np.float32))
    s = np.ascontiguousarray(np.asarray(s, dtype=np.float32))
    diag = np.einsum("ij,ij->i", im, s).astype(np.float32)
    imT16 = np.ascontiguousarray(im.T.astype(np.float16))
    sT16_full = np.ascontiguousarray(s.T.astype(np.float16))
    eye16 = np.eye(128, dtype=np.float16)
    negeye16 = (eye16 * np.float16(MASK)).astype(np.float16)
    in_maps = []
    for r in range(NCORES):
        lo = r * RL
        rolled_diag = np.roll(diag, -lo)
        in_maps.append({
            "imT": np.ascontiguousarray(imT16[:, lo:lo + RL]),
            "sT": np.ascontiguousarray(np.roll(sT16_full, -lo, axis=1)),
            "diag_r": np.ascontiguousarray(diag[lo:lo + RL].reshape(NT, 128).T),
            "diag_c": np.ascontiguousarray(rolled_diag.reshape(NCC, 128).T),
            "eye16": eye16,
            "negeye16": negeye16,
        })
    return in_maps, diag


def finish(results, diag):
    """Host-side reduction of the per-core stats to the scalar loss."""
    diag64 = diag.astype(np.float64)
    total = 0.0
    s2_sum = np.zeros(N, dtype=np.float64)
    cmax_g = np.full(N, -np.inf, dtype=np.float64)
    for r in range(NCORES):
        lo = r * RL
        s1 = results[r]["s1"].astype(np.float64)    # [128, NT*NSC]
        s2 = results[r]["s2"].astype(np.float64)    # [128, NCC] sign sums
        rmax = results[r]["rmax"].astype(np.float64)
        cmax = results[r]["cmax"].astype(np.float64)  # [128, N] (fp16 in)
        # s1: block (t, sc) in column t*NSC+sc. t==0 blocks hold direct
        # DVE is_lt counts; t>0 hold ACT sign sums -> (1024+S)/2.
        s1b = s1.reshape(128, NT, NSC)
        cnt_blk = np.empty_like(s1b)
        cnt_blk[:, 0, :] = s1b[:, 0, :]
        cnt_blk[:, 1:, :] = (SC_W + s1b[:, 1:, :]) / 2.0
        cnt1 = cnt_blk.sum(axis=2).T.reshape(RL)    # = rank1 + 1
        rmaxv = rmax.reshape(128, NT, NSC).max(axis=2).T.reshape(RL)
        d_loc = diag64[lo:lo + RL]
        total += np.sum(np.maximum(MARGIN + rmaxv - d_loc, 0.0) / cnt1)
        # columns: rotated col j' = cc*128+p -> global j = (lo + j') % N
        jj = (lo + np.arange(N)) % N
        s2_sum[jj] += s2.T.reshape(N)               # sign sums over rows
        cmax_g[jj] = np.maximum(cmax_g[jj], cmax.T.reshape(N))
    cnt2 = (N + s2_sum) / 2.0                       # = rank2 + 1
    total += np.sum(np.maximum(MARGIN + cmax_g - diag64, 0.0) / cnt2)
    return np.array(total, dtype=np.float32)


def run_on_hw(im, s, trace=False):
    from concourse.bass_utils import run_bass_kernel_spmd

    in_maps, diag = make_in_maps(im, s)
    nc = _get_nc()
    out = run_bass_kernel_spmd(nc, in_maps, list(range(NCORES)), trace=trace)
    return finish(out.results, diag), out


def kernel(im, s):
    result, _ = run_on_hw(im, s, trace=False)
    return result


# revision 30
# speedup vs baseline: 2.1656x; 1.0341x over previous
"""Trainium2 Bass kernel for the rank-weighted hard-negative hinge loss.

Math (reference):
    scores = im @ s.T                         # [N, N]
    diag   = diagonal(scores)
    rank1[i] = #{j : scores[i,j] < diag[i]}   (row rank of diag)
    rank2[j] = #{i : scores[i,j] < diag[j]}   (col rank of diag)
    cost_s  = 1/(rank1+1) * max_j!=i relu(M + scores[i,j] - diag[i])
    cost_im = 1/(rank2+1) * max_i!=j relu(M + scores[i,j] - diag[j])
    loss = sum(cost_s) + sum(cost_im)

v3 "dual orientation" design:
  - scores in fp16 (1 cyc/row on PE vs 4 for fp32; verified rel err ~6e-4
    against the 2e-2 gate, with the diagonal masked deterministically so
    rank counts are exact in structure: cnt = rank+1).
  - PE computes each score block TWICE: row-major ps[row, col] and
    transposed psT[col, row] (from the same SBUF-resident fp16 inputs).
    This makes BOTH rank counts per-partition-threshold Sign+accum ops
    on the ACT engine, removing the indicator tensors, count matmuls
    and the [128,N] diag broadcast of the fp32 baseline.
  - per block: DVE runs ONE fused tensor_tensor_reduce over ps that
    writes H = fp16(ps) to SBUF AND row-max-accumulates (rmax), then a
    2x-packed fp16 max accumulate for the running column max. rank1 for
    one of 8 row tiles runs as a 4x-packed tensor_scalar(is_lt)+accum
    on DVE to balance ACT vs DVE load. Pool does the t==0 column-max
    copies. Everything else is ACT Sign+accum.
  - host folds the tiny per-core stats (rank sums, rmax cols, cmax
    partitions) and reduces across the 8 cores.

Sharding: core r owns rows [r*1024, (r+1)*1024); s.T columns are rotated
left by r*1024 so the diagonal block sits at local column offset = local
row index on every core (single SPMD program). Host un-rotates.
"""

import numpy as np

N = 8192
D = 256
NCORES = 8
RL = N // NCORES  # rows per core
MARGIN = 0.2
MASK = np.float32(-30000.0)  # diag mask offset; fp16-safe sentinel

SC_W = 1024            # column superchunk width
NSC = N // SC_W        # 8 superchunks
NT = RL // 128         # 8 row tiles
NCC = N // 128         # 64 psT col chunks

_cache = {}


def _build_nc():
    import concourse.bacc as bacc
    import concourse.mybir as mybir
    from concourse.tile import TileContext

    f32 = mybir.dt.float32
    f16 = mybir.dt.float16

    Sign = mybir.ActivationFunctionType.Sign
    AX = mybir.AxisListType.X
    MAX = mybir.AluOpType.max
    ADD = mybir.AluOpType.add
    MULT = mybir.AluOpType.mult
    LT = mybir.AluOpType.is_lt

    nc = bacc.Bacc(None)

    imT = nc.declare_dram_parameter("imT", [D, RL], f16, isOutput=False)
    sT = nc.declare_dram_parameter("sT", [D, N], f16, isOutput=False)
    diag_r = nc.declare_dram_parameter("diag_r", [128, NT], f32, isOutput=False)
    diag_c = nc.declare_dram_parameter("diag_c", [128, NCC], f32, isOutput=False)
    eye16 = nc.declare_dram_parameter("eye16", [128, 128], f16, isOutput=False)
    negeye16 = nc.declare_dram_parameter("negeye16", [128, 128], f16, isOutput=False)
    s1_o = nc.declare_dram_parameter("s1", [128, NT * NSC], f32, isOutput=True)
    s2_o = nc.declare_dram_parameter("s2", [128, NCC], f32, isOutput=True)
    rmax_o = nc.declare_dram_parameter("rmax", [128, NT * NSC], f32, isOutput=True)
    cmax_o = nc.declare_dram_parameter("cmax", [128, NCC], f32, isOutput=True)

    with TileContext(nc) as tc:
        with (
            tc.tile_pool(name="consts", bufs=1) as cpool,
            tc.tile_pool(name="data", bufs=1) as dpool,
            tc.tile_pool(name="ps", bufs=2, space="PSUM") as pspool,
            tc.tile_pool(name="psT", bufs=2, space="PSUM") as pstpool,
            tc.tile_pool(name="trash", bufs=3) as tpool,
            tc.tile_pool(name="outs", bufs=1) as opool,
        ):
            t_eye16 = cpool.tile([128, 128], f16, tag="eye16")
            nc.sync.dma_start(out=t_eye16[:], in_=eye16[:])
            t_negeye16 = cpool.tile([128, 128], f16, tag="negeye16")
            nc.sync.dma_start(out=t_negeye16[:], in_=negeye16[:])
            t_dr = cpool.tile([128, NT], f32, tag="dr")
            nc.sync.dma_start(out=t_dr[:], in_=diag_r[:])
            t_dc = cpool.tile([128, NCC], f32, tag="dc")
            nc.sync.dma_start(out=t_dc[:], in_=diag_c[:])
            t_imT = []
            for k in range(2):
                t = dpool.tile([128, RL], f16, tag=f"imT{k}")
                nc.sync.dma_start(out=t[:], in_=imT[k * 128:(k + 1) * 128, :])
                t_imT.append(t)
            t_sT = {}
            for b in range(NSC):
                for k in range(2):
                    t = dpool.tile([128, SC_W], f16, tag=f"sT{k}_{b}")
                    nc.sync.dma_start(
                        out=t[:],
                        in_=sT[k * 128:(k + 1) * 128, b * SC_W:(b + 1) * SC_W],
                    )
                    t_sT[(k, b)] = t

            t_s1 = opool.tile([128, NT * NSC], f32, tag="s1")
            t_s2 = opool.tile([128, NCC], f32, tag="s2")
            t_rmax = opool.tile([128, NT * NSC], f32, tag="rmax")
            t_cmax = opool.tile([128, NCC], f32, tag="cmax")

            for sc in range(NSC):
                for t in range(NT):
                    idx = t * NSC + sc
                    cc = sc * NT + t  # psT col chunk handled this iteration

                    # sc==0 blocks contain the (rotated) diagonal at free
                    # offset t*128; mask it with an extra accumulating
                    # matmul  ps += I^T @ (-30000*I)  inside the group so
                    # no vector-engine pass (or extra dependency) is needed.
                    off = t * 128
                    cm = off // 512  # 512-wide region holding the diagonal
                    ps = pspool.tile([128, SC_W], f32, tag="ps")
                    for k in range(2):
                        for c in range(SC_W // 512):
                            nc.tensor.matmul(
                                ps[:, c * 512:(c + 1) * 512],
                                lhsT=t_imT[k][:, t * 128:(t + 1) * 128],
                                rhs=t_sT[(k, sc)][:, c * 512:(c + 1) * 512],
                                start=(k == 0),
                                stop=(k == 1) and not (sc == 0 and c == cm),
                            )
                    if sc == 0:
                        nc.tensor.matmul(
                            ps[:, off:off + 128],
                            lhsT=t_eye16[:], rhs=t_negeye16[:],
                            start=False, stop=True, skip_group_check=True,
                        )
                    psT = pstpool.tile([128, RL], f32, tag="psT")
                    for k in range(2):
                        for c in range(RL // 512):
                            nc.tensor.matmul(
                                psT[:, c * 512:(c + 1) * 512],
                                lhsT=t_sT[(k, sc)][:, t * 128:(t + 1) * 128],
                                rhs=t_imT[k][:, c * 512:(c + 1) * 512],
                                start=(k == 0),
                                stop=(k == 1) and not (sc == 0 and c == cm),
                            )
                    if sc == 0:
                        # psT diag: col cc*128+p is at row (free) cc*128+p
                        nc.tensor.matmul(
                            psT[:, off:off + 128],
                            lhsT=t_eye16[:], rhs=t_negeye16[:],
                            start=False, stop=True, skip_group_check=True,
                        )
                    # row max straight from PSUM (no fp16 staging copy: the
                    # DVE 2x/4x packed modes do not engage on this HW, so a
                    # copy costs a full ACT pass and buys nothing)
                    nc.vector.tensor_reduce(
                        t_rmax[:, idx:idx + 1], ps[:], AX, MAX)
                    # rank2: sign(diag_col - psT), accumulated over rows
                    trash2 = tpool.tile([128, SC_W], f16, tag="trash2")
                    nc.scalar.activation(
                        trash2[:], psT[:], Sign,
                        bias=t_dc[:, cc:cc + 1], scale=-1.0,
                        accum_out=t_s2[:, cc:cc + 1],
                    )
                    # rank1: ACT Sign+accum on ps (sign sums); half the
                    # t==0 tiles run on DVE (is_lt + add-reduce accum ->
                    # direct count) to balance ACT vs DVE load.
                    trash1 = tpool.tile([128, SC_W], f16, tag="trash1")
                    if t == 0 and sc % 2 == 0:
                        nc.vector.tensor_scalar(
                            trash1[:], ps[:], t_dr[:, t:t + 1], 0.0, LT,
                            ADD, accum_out=t_s1[:, idx:idx + 1],
                        )
                    else:
                        nc.scalar.activation(
                            trash1[:], ps[:], Sign,
                            bias=t_dr[:, t:t + 1], scale=-1.0,
                            accum_out=t_s1[:, idx:idx + 1],
                        )
                    # column max over this core's rows, straight from the
                    # transposed block (free-axis reduce, [128,1] per chunk)
                    nc.vector.tensor_reduce(
                        t_cmax[:, cc:cc + 1], psT[:], AX, MAX)

            nc.sync.dma_start(out=s1_o[:], in_=t_s1[:])
            nc.sync.dma_start(out=s2_o[:], in_=t_s2[:])
            nc.sync.dma_start(out=rmax_o[:], in_=t_rmax[:])
            nc.sync.dma_start(out=cmax_o[:], in_=t_cmax[:])

    nc.finalize()
    return nc


def _get_nc():
    if "nc" not in _cache:
        _cache["nc"] = _build_nc()
    return _cache["nc"]


def make_in_maps(im, s):
    im = np.ascontiguousarray(np.asarray(im, dtype=np.float32))
    s = np.ascontiguousarray(np.asarray(s, dtype=np.float32))
    diag = np.einsum("ij,ij->i", im, s).astype(np.float32)
    imT16 = np.ascontiguousarray(im.T.astype(np.float16))
    sT16_full = np.ascontiguousarray(s.T.astype(np.float16))
    eye16 = np.eye(128, dtype=np.float16)
    negeye16 = (eye16 * np.float16(MASK)).astype(np.float16)
    in_maps = []
    for r in range(NCORES):
        lo = r * RL
        rolled_diag = np.roll(diag, -lo)
        in_maps.append({
            "imT": np.ascontiguousarray(imT16[:, lo:lo + RL]),
            "sT": np.ascontiguousarray(np.roll(sT16_full, -lo, axis=1)),
            "diag_r": np.ascontiguousarray(diag[lo:lo + RL].reshape(NT, 128).T),
            "diag_c": np.ascontiguousarray(rolled_diag.reshape(NCC, 128).T),
            "eye16": eye16,
            "negeye16": negeye16,
        })
    return in_maps, diag


def finish(results, diag):
    """Host-side reduction of the per-core stats to the scalar loss."""
    diag64 = diag.astype(np.float64)
    total = 0.0
    s2_sum = np.zeros(N, dtype=np.float64)
    cmax_g = np.full(N, -np.inf, dtype=np.float64)
    for r in range(NCORES):
        lo = r * RL
        s1 = results[r]["s1"].astype(np.float64)    # [128, NT*NSC]
        s2 = results[r]["s2"].astype(np.float64)    # [128, NCC] sign sums
        rmax = results[r]["rmax"].astype(np.float64)
        cmax = results[r]["cmax"].astype(np.float64)  # [128, N] (fp16 in)
        # s1: block (t, sc) in column t*NSC+sc. (t==0, even sc) blocks
        # hold direct DVE is_lt counts; the rest hold ACT sign sums
        # -> (1024+S)/2.
        s1b = s1.reshape(128, NT, NSC)
        cnt_blk = (SC_W + s1b) / 2.0
        cnt_blk[:, 0, 0::2] = s1b[:, 0, 0::2]
        cnt1 = cnt_blk.sum(axis=2).T.reshape(RL)    # = rank1 + 1
        rmaxv = rmax.reshape(128, NT, NSC).max(axis=2).T.reshape(RL)
        d_loc = diag64[lo:lo + RL]
        total += np.sum(np.maximum(MARGIN + rmaxv - d_loc, 0.0) / cnt1)
        # columns: rotated col j' = cc*128+p -> global j = (lo + j') % N
        jj = (lo + np.arange(N)) % N
        s2_sum[jj] += s2.T.reshape(N)               # sign sums over rows
        cmax_g[jj] = np.maximum(cmax_g[jj], cmax.T.reshape(N))
    cnt2 = (N + s2_sum) / 2.0                       # = rank2 + 1
    total += np.sum(np.maximum(MARGIN + cmax_g - diag64, 0.0) / cnt2)
    return np.array(total, dtype=np.float32)


def run_on_hw(im, s, trace=False):
    from concourse.bass_utils import run_bass_kernel_spmd

    in_maps, diag = make_in_maps(im, s)
    nc = _get_nc()
    out = run_bass_kernel_spmd(nc, in_maps, list(range(NCORES)), trace=trace)
    return finish(out.results, diag), out


def kernel(im, s):
    result, _ = run_on_hw(im, s, trace=False)
    return result
